# revision 42
# baseline (speedup 1.0000x reference)
"""Trainium2 Bass kernel for AudioGRUModel: GRU over 256 steps, final hidden.

Strategy: 8-way data-parallel over batch (32 rows/core), weights replicated.
All on-chip layouts are transposed ([feature-dim on partitions, batch on free])
so the sequential recurrence needs no per-step transposes.

The serial gate chain, not the weight stream, is the bottleneck (the 48
weight matmuls issue at ~25ns each), so everything here attacks the
per-step dependency chain (1442us baseline -> 905us):

* gi (input projection) lives in an SBUF window (bf16), never round-trips
  through DRAM. The per-step G load + fp32 "s1 = p_rz + G" DVE add are
  gone: an identity-stationary selector matmul accumulates G straight into
  the PSUM banks, so the sigmoids read PSUM directly.
* h and all gates are bf16 (verified: rel err 0.008 vs the 2e-2 gate).
* r, z, n get separate PSUM banks with separate accumulation groups — Tile
  treats a whole group as one write, so a fused bank would make the r
  sigmoid wait for the z matmuls too.
* blend is h' = tanh(n)*zc + z*h with zc = 1-z on the DVE (STT op) and
  zh = z*h computed while the tanh runs; the next step's r matmuls consume
  zh and u = tanh(n)*zc as two moving operands (PSUM adds them), so the
  r-stream starts before the h' add, and the zh half streams even earlier.
* explicit engine-FIFO order chains on ACT/DVE (mirroring the PE "mm"
  chain): the Tile scheduler otherwise interleaves projection evacuations
  into the chain (measured: tanh stalled ~850ns behind an evac).
* bhhn/sel selector padded to K=128 — a K=4 stationary is a partial
  row-group LDWEIGHTS, which drains the PE pipeline mid-stream.
* x is host-rearranged to [INP, slab, BL, SQ] so slab DMAs are contiguous;
  slab transposes are chunked per (k, group) and ride the DVE idle window;
  the final h is written transposed (contiguous DMA, ~75us cheaper) and
  transposed back on the host.
"""

import numpy as np
import ml_dtypes

import concourse.bass as bass
import concourse.tile as tile
from concourse import mybir, bacc
from concourse.tile import add_dep_helper
from concourse.bass_utils import run_bass_kernel_spmd

F32 = mybir.dt.float32
BF16 = mybir.dt.bfloat16
AF = mybir.ActivationFunctionType

B, INP, S, H = 256, 512, 256, 512
G3 = 3 * H            # 1536
NC = 8
BL = B // NC          # 32 batch rows per core
KC = H // 128         # 4 contraction chunks
MC = G3 // 128        # 12 output chunks (0-3 r, 4-7 z, 8-11 n)
SQ = 64               # steps per x-staging slab
SG = 16               # steps per 512-col projection group
LEAD = 1              # projection groups kept ahead of the recurrence


def _dedup_ldweights(nc):
    """Remove LDWEIGHTS that reload the exact weights already resident."""
    removed = 0
    for f in nc.m.functions:
        for bb in f.blocks:
            insts = bb.instructions
            del_ids = set()
            last_key = None
            for i in insts:
                if type(i).__name__ == 'InstLdweights':
                    a = i.ins[0]
                    k = (a.memref, a.offset, str(a.ap), str(a.dtype),
                         str(i.perf_mode), str(i.tile_position))
                    has_sync = bool(i.sync_info and
                                    (i.sync_info.on_wait or i.sync_info.on_update))
                    if k == last_key and not has_sync:
                        del_ids.add(id(i))
                        continue
                    last_key = k
            if del_ids:
                insts[:] = [i for i in insts if id(i) not in del_ids]
            removed += len(del_ids)
    return removed


def _build(steps=S):
    nc = bacc.Bacc("TRN2", target_bir_lowering=False, debug=False)

    # x arrives host-rearranged to [INP, n_slabs, BL, SQ] so each slab DMA
    # reads 4KB-contiguous runs per partition (128 descriptors, not 4096)
    nslab = (steps + SQ - 1) // SQ
    xb_d = nc.dram_tensor("x_bf", [INP, nslab, BL, SQ], BF16,
                          kind="ExternalInput")
    wih_d = nc.dram_tensor("wih_t", [INP, G3], BF16, kind="ExternalInput")
    whh_d = nc.dram_tensor("whh_t", [H, G3], BF16, kind="ExternalInput")
    # bhhn/sel32 padded to K=128: a K=4 stationary would be a partial
    # row-group LDWEIGHTS, which stalls the PE pipeline mid-stream
    bsum_d = nc.dram_tensor("bsum", [128, MC], F32, kind="ExternalInput")
    bhhn_d = nc.dram_tensor("bhhn", [128, 128], BF16, kind="ExternalInput")
    sel_d = nc.dram_tensor("sel32", [128, 128], BF16, kind="ExternalInput")
    id_d = nc.dram_tensor("ident", [128, 128], BF16, kind="ExternalInput")
    # output stays transposed ([H, BL]) so the final DMA is contiguous;
    # the host transposes (a [b p -> p b] scatter DMA here cost ~75us)
    out_d = nc.dram_tensor("h_out", [H, BL], F32, kind="ExternalOutput")

    all_mms = []

    def mm(*args, **kwargs):
        m = nc.tensor.matmul(*args, **kwargs)
        if all_mms:
            add_dep_helper(m.ins, all_mms[-1].ins, False, "pe-order")
        all_mms.append(m)
        return m

    # Force engine-FIFO order to match emission order on ACT and DVE too —
    # the Tile scheduler otherwise interleaves projection evacuations into
    # the serial gate chain (measured: tanh stalled ~850ns behind an evac).
    last_act = []
    last_dve = []

    def act(fn, *args, **kwargs):
        i = fn(*args, **kwargs)
        if last_act:
            add_dep_helper(i.ins, last_act[0].ins, False, "act-order")
        last_act[:] = [i]
        return i

    def dve(fn, *args, **kwargs):
        i = fn(*args, **kwargs)
        if last_dve:
            add_dep_helper(i.ins, last_dve[0].ins, False, "dve-order")
        last_dve[:] = [i]
        return i

    ngroups = steps // SG

    with tile.TileContext(nc) as tc:
        with (
            tc.tile_pool(name="consts", bufs=1) as consts,
            tc.tile_pool(name="xstage", bufs=2) as xstage,
            tc.tile_pool(name="xtr", bufs=2) as xtrp,
            tc.tile_pool(name="win", bufs=3) as winp,
            tc.tile_pool(name="ipsum", bufs=2, space="PSUM") as ipsum,
            tc.tile_pool(name="pr", bufs=2, space="PSUM") as prp,
            tc.tile_pool(name="pz", bufs=2, space="PSUM") as pzp,
            tc.tile_pool(name="pn", bufs=2, space="PSUM") as pnp,
            tc.tile_pool(name="gates", bufs=2) as gates,
        ):
            # ---- constants / weights ----
            # DMA issue order matters at startup (the Sync queue issues them
            # serially): slab 0 + wih first (they gate the first projection),
            # whh last (first needed by step 1's matmuls, ~25us in)
            wih = consts.tile([128, KC, G3], BF16)
            whh = consts.tile([128, KC, G3], BF16)
            bsum = consts.tile([128, MC], F32)
            nc.sync.dma_start(out=bsum[:], in_=bsum_d.ap())
            bhhn = consts.tile([128, 128], BF16)
            nc.sync.dma_start(out=bhhn[:], in_=bhhn_d.ap())
            sel32 = consts.tile([128, 128], BF16)
            nc.sync.dma_start(out=sel32[:], in_=sel_d.ap())
            ident = consts.tile([128, 128], BF16)
            nc.sync.dma_start(out=ident[:], in_=id_d.ap())
            ones = consts.tile([128, 128], BF16)
            nc.vector.memset(ones[:], 1.0)
            zer512 = consts.tile([128, 512], BF16)
            nc.vector.memset(zer512[:], 0.0)

            # h state, bf16, ping-pong buffers
            hb = [consts.tile([128, 128], BF16, name=f"hb{i}") for i in range(2)]
            nc.vector.memset(hb[0][:], 0.0)
            nc.vector.memset(hb[1][:], 0.0)

            # ---- input-projection machinery (emitted incrementally) ----
            # gi window tiles: [128, MC, SG, BL] bf16, one per 16-step group
            slab_tiles = {}
            win_tiles = {}

            def stage_slab(q):
                xt = xstage.tile([128, KC, BL, SQ], BF16, name="xt", tag="xt")
                xt3 = xtrp.tile([128, KC, SQ, BL], BF16, name="xt3", tag="xt3")
                for k in range(KC):
                    nc.sync.dma_start(
                        out=xt[:, k, :, :],
                        in_=xb_d[128 * k:128 * (k + 1), q, :, :],
                    )
                slab_tiles[q] = (xt, xt3)

            ip_state = {}

            def iproj_mm(g, j):
                """Emit the j-th projection matmul (of 48) for step-group g."""
                m_, k = j // KC, j % KC
                xt, xt3 = slab_tiles[g // (SQ // SG)]
                goff = (g % (SQ // SG)) * SG
                if j == 0:
                    win_tiles[g] = winp.tile([128, MC, SG, BL], BF16,
                                             name="win", tag="win")
                if k == 0:
                    ip_state[g] = ipsum.tile([128, SG * BL], F32,
                                             name="ips", tag="ips")
                ps = ip_state[g]
                mm(ps[:], wih[:, k, 128 * m_:128 * (m_ + 1)],
                   xt3[:, k, goff:goff + SG, :],
                   start=(k == 0), stop=(k == KC - 1))
                if k == KC - 1:
                    # evacuate with bias straight into the bf16 SBUF window.
                    # For the up-front group the 12 evacs would serialize on
                    # ACT (~8us of startup); alternate them onto the DVE
                    # ((ps + bias) + 0 via scalar_tensor_tensor) so the two
                    # engines drain the projection in parallel.
                    if g < LEAD and m_ % 2 == 1:
                        dve(nc.vector.scalar_tensor_tensor,
                            win_tiles[g][:, m_, :, :], ps[:],
                            bsum[:, m_:m_ + 1], zer512[:],
                            mybir.AluOpType.add, mybir.AluOpType.add)
                    else:
                        act(nc.scalar.activation,
                            win_tiles[g][:, m_, :, :], ps[:], AF.Identity,
                            bias=bsum[:, m_:m_ + 1], scale=1.0)

            def xcopy(g2, k):
                """Transpose [b,s]->[s,b] for group g2, contraction chunk k."""
                xt, xt3 = slab_tiles[g2 // (SQ // SG)]
                goff = (g2 % (SQ // SG)) * SG
                dve(nc.vector.tensor_copy,
                    xt3[:, k, goff:goff + SG, :],
                    xt[:, k, :, goff:goff + SG].rearrange("p b s -> p s b"))

            # up-front: first slab, transposes for groups 0..LEAD, and the
            # LEAD groups fully projected
            stage_slab(0)
            for k in range(KC):
                nc.sync.dma_start(out=wih[:, k, :],
                                  in_=wih_d[128 * k:128 * (k + 1), :])
            for k in range(KC):
                nc.sync.dma_start(out=whh[:, k, :],
                                  in_=whh_d[128 * k:128 * (k + 1), :])
            for g in range(min(LEAD + 1, ngroups)):
                for k in range(KC):
                    xcopy(g, k)
            up = min(LEAD, ngroups)
            for g in range(up):
                for m_ in range(MC):
                    for k in range(KC):
                        iproj_mm(g, m_ * KC + k)

            # ---- recurrence with interleaved projection ----
            for t in range(steps):
                # stage slab q a full slab-window ahead of its first use
                for q in range(1, (steps + SQ - 1) // SQ):
                    if t == SQ * (q - 1):
                        stage_slab(q)

                win = win_tiles[t // SG]
                toff = t % SG
                h_in = hb[t % 2]
                h_out = hb[(t + 1) % 2]

                # --- PE: r bank first. Instead of waiting for h = u + zh, the
                # r matmuls consume zh and u as separate moving operands (PSUM
                # accumulates), so the zh half streams during the chain tail
                # and the u half fires straight off the u multiply.
                p_r = prp.tile([128, 128], F32, name="pr", tag="pr")
                mm(p_r[:], ident[:], win[:, 0:4, toff, :],
                   start=True, stop=(t == 0))
                if t > 0:
                    for src in (zh_prev, u_prev):
                        for m_ in range(4):
                            for k in range(KC):
                                mm(p_r[:, 32 * m_:32 * (m_ + 1)],
                                   whh[:, k, 128 * m_:128 * (m_ + 1)],
                                   src[:, 32 * k:32 * (k + 1)],
                                   start=False,
                                   stop=(src is u_prev) and (m_ == 3)
                                   and (k == KC - 1))
                # n bank next, so tt = r*p_n isn't starved
                p_n = pnp.tile([128, 128], F32, name="pn", tag="pn")
                mm(p_n[:], bhhn[:], sel32[:], start=True, stop=(t == 0))
                if t > 0:
                    for m_ in range(8, MC):
                        c0 = 32 * (m_ - 8)
                        for k in range(KC):
                            mm(p_n[:, c0:c0 + 32],
                               whh[:, k, 128 * m_:128 * (m_ + 1)],
                               h_in[:, 32 * k:32 * (k + 1)],
                               start=False,
                               stop=(m_ == MC - 1) and (k == KC - 1))
                # z matmuls last (z is only needed late, for zc/zh)
                p_z = pzp.tile([128, 128], F32, name="pz", tag="pz")
                mm(p_z[:], ident[:], win[:, 4:8, toff, :],
                   start=True, stop=(t == 0))
                if t > 0:
                    for m_ in range(4, 8):
                        for k in range(KC):
                            mm(p_z[:, 32 * (m_ - 4):32 * (m_ - 3)],
                               whh[:, k, 128 * m_:128 * (m_ + 1)],
                               h_in[:, 32 * k:32 * (k + 1)],
                               start=False,
                               stop=(m_ == 7) and (k == KC - 1))

                # --- gate chain (ACT: sig_r, sig_z, tanh; DVE: the rest) ---
                r = gates.tile([128, 128], BF16, name="r", tag="r")
                act(nc.scalar.activation, r[:], p_r[:], AF.Sigmoid)
                z = gates.tile([128, 128], BF16, name="z", tag="z")
                act(nc.scalar.activation, z[:], p_z[:], AF.Sigmoid)

                tt = gates.tile([128, 128], BF16, name="tt", tag="tt")
                dve(nc.vector.tensor_mul, tt[:], r[:], p_n[:])
                vv = gates.tile([128, 128], BF16, name="vv", tag="vv")
                dve(nc.vector.tensor_add, vv[:], tt[:], win[:, 8:12, toff, :])
                nn = gates.tile([128, 128], BF16, name="nn", tag="nn")
                act(nc.scalar.activation, nn[:], vv[:], AF.Tanh)

                zc = gates.tile([128, 128], BF16, name="zc", tag="zc")
                dve(nc.vector.scalar_tensor_tensor, zc[:], z[:], -1.0, ones[:],
                    mybir.AluOpType.mult, mybir.AluOpType.add)
                zh = gates.tile([128, 128], BF16, name="zh", tag="zh")
                dve(nc.vector.tensor_mul, zh[:], z[:], h_in[:])
                u = gates.tile([128, 128], BF16, name="u", tag="u")
                dve(nc.vector.tensor_mul, u[:], nn[:], zc[:])
                dve(nc.vector.tensor_add, h_out[:], u[:], zh[:])
                u_prev, zh_prev = u, zh

                # --- off-path work: x transposes for group t//SG + LEAD + 1,
                # projection matmuls + evac for group t//SG + LEAD
                g2 = t // SG + LEAD + 1
                if toff < KC and g2 < ngroups:
                    xcopy(g2, toff)
                g = t // SG + LEAD
                if g < ngroups:
                    j0 = 3 * toff
                    for j in (j0, j0 + 1, j0 + 2):
                        iproj_mm(g, j)

            # ---- output: cast to fp32 and un-transpose h^T -> h ----
            hf = consts.tile([128, 128], F32, name="hf")
            dve(nc.vector.tensor_copy, hf[:], hb[steps % 2][:])
            for k in range(KC):
                nc.sync.dma_start(
                    out=out_d[128 * k:128 * (k + 1), :],
                    in_=hf[:, 32 * k:32 * (k + 1)],
                )

    nc.compile()
    _dedup_ldweights(nc)
    return nc


def _prep_inputs(x, weight_ih, weight_hh, bias_ih, bias_hh):
    x = np.ascontiguousarray(np.asarray(x, dtype=np.float32))
    w_ih = np.asarray(weight_ih, dtype=np.float32)
    w_hh = np.asarray(weight_hh, dtype=np.float32)
    b_ih = np.asarray(bias_ih, dtype=np.float32)
    b_hh = np.asarray(bias_hh, dtype=np.float32)

    x_bf = x.astype(ml_dtypes.bfloat16)
    wih_t = np.ascontiguousarray(w_ih.T).astype(ml_dtypes.bfloat16)
    whh_t = np.ascontiguousarray(w_hh.T).astype(ml_dtypes.bfloat16)
    bsum = np.empty((128, MC), np.float32)
    for m in range(MC):
        seg = b_ih[128 * m:128 * (m + 1)].copy()
        if m < 8:
            seg += b_hh[128 * m:128 * (m + 1)]
        bsum[:, m] = seg
    bhhn = np.zeros((128, 128), np.float32)
    bhhn[:KC] = b_hh[2 * H:].reshape(KC, 128)
    bhhn = bhhn.astype(ml_dtypes.bfloat16)
    sel32 = np.zeros((128, 128), np.float32)
    for k in range(KC):
        sel32[k, 32 * k:32 * (k + 1)] = 1.0
    sel32 = sel32.astype(ml_dtypes.bfloat16)
    ident = np.eye(128, dtype=np.float32).astype(ml_dtypes.bfloat16)

    shared = {"wih_t": wih_t, "whh_t": whh_t, "bsum": bsum,
              "bhhn": bhhn, "sel32": sel32, "ident": ident}
    in_maps = []
    for c in range(NC):
        m = dict(shared)
        xc = x_bf[BL * c:BL * (c + 1)].transpose(1, 0, 2)      # [INP, BL, S]
        xc = xc.reshape(INP, BL, S // SQ, SQ).transpose(0, 2, 1, 3)
        m["x_bf"] = np.ascontiguousarray(xc)                   # [INP, q, BL, SQ]
        in_maps.append(m)
    return in_maps


_NC_CACHE = {}


def _get_nc(steps=S):
    if steps not in _NC_CACHE:
        _NC_CACHE[steps] = _build(steps)
    return _NC_CACHE[steps]


def kernel(x, weight_ih, weight_hh, bias_ih, bias_hh):
    nc = _get_nc(S)
    in_maps = _prep_inputs(x, weight_ih, weight_hh, bias_ih, bias_hh)
    res = run_bass_kernel_spmd(nc, in_maps, core_ids=list(range(NC)))
    return np.concatenate(
        [np.asarray(res.results[c]["h_out"]).T for c in range(NC)], axis=0
    ).astype(np.float32)
